# revision 2
# baseline (speedup 1.0000x reference)
"""CLUB loss kernel for 8x TRN2 NeuronCores.

Math: per sample b (L=512 positions, D=64 dims):
  mu     = MLP_mu(x);  logvar = tanh(MLP_lv(x));  iv = exp(-logvar)
  loss = -0.5/(B*L) * sum_{b,d,l} [ ((ysq - Ey2) - mu*yd2) * iv ]
with ysq = y^2, yd2 = 2*(y - Ey); Ey/Ey2 are per-(b,d) means over l.

y never feeds a matmul, so ysq/yd2/Ey2 are precomputed host-side and
shipped (bf16) instead of y. On-chip work per core (1 sample):
  - 2-layer MLP for lv and mu paths, chunked over L halves
  - ACT spine: relu_lv -> tanh -> exp(-.) (+ free sum_l iv accumulator)
  - DVE: relu_mu; m2 = (mu_nb + b2mu) * yd2 written in place over yd2
    (rows 64:128); fin = [ysq; m2] * [iv; iv] as ONE [128, HC] op with
    per-partition accumulators (rows 0:64 accumulate +ysq*iv terms,
    rows 64:128 the m2*iv terms)
  - gpsimd duplicates iv into rows 64:128 for the stacked fin op
  - one fp32 collapse matmul acct^T @ [±1 | -Ey2] turns the accumulator
    columns into the 4 scalars the host sums.

All MLP matmul operands are bf16 (fp32 PE runs 4 cyc/col and fp32r
truncates to ~bf16 anyway); PSUM/elementwise stay fp32. ysq/yd2 in bf16
add ~1e-5 relative error (random-sign rounding across 32K terms).

DMA: two packed [128, W] bf16 input tensors (one descriptor per
partition, few large descriptors per queue) triggered on SP and ACT
HWDGE; no SWDGE. f32 constants (biases, collapse vectors) ride in the
bf16 pack and are bitcast back on-chip. The output store is shaped
[16, 2] so its 16 descriptors touch all 16 DMA engines: a DMA's
completion semaphore gets 16 increments (one per engine) and engines
with zero descriptors only bump on a lazy ~5-8us sweep, which is what
made the old single-packet store stall the end-of-NEFF barrier.

Sharding: data-parallel over batch B=8, one sample per core; host does
the tiny final combine.
"""

import sys

if "/opt/trn_rl_repo" not in sys.path:
    sys.path.insert(0, "/opt/trn_rl_repo")

import numpy as np

B, L = 8, 512
XD, YD, H = 192, 64, 128
NCORES = 8
HC = L // 2

WA = 1024  # pkA: w1 packs + xa0/xb0
WB = 1162  # pkB: xa1/xb1 + w2 packs + u blocks + f32 consts

_CACHE: dict = {}


def build_nc(debug: bool = False):
    import concourse.bass as bass
    import concourse.bacc as bacc
    import concourse.tile as tile
    from concourse import mybir
    from concourse.tile import add_dep_helper

    f32 = mybir.dt.float32
    bf16 = mybir.dt.bfloat16
    AF = mybir.ActivationFunctionType
    OP = mybir.AluOpType

    nc = bacc.Bacc("TRN2", target_bir_lowering=False, debug=debug)

    pa_d = nc.dram_tensor("pa", [128, WA], bf16, kind="ExternalInput")
    pb_d = nc.dram_tensor("pb", [128, WB], bf16, kind="ExternalInput")
    acc_d = nc.dram_tensor("acc", [16, 2], f32, kind="ExternalOutput")

    with tile.TileContext(nc) as tc:
        with (
            tc.tile_pool(name="sb", bufs=1) as sb,
            tc.tile_pool(name="ps", bufs=1, space=bass.MemorySpace.PSUM) as ps,
        ):
            # input DMAs first: pkA (gates the first matmuls) on SP,
            # pkB on ACT; both HWDGE, 128 descriptors each.
            pa = sb.tile([128, WA], bf16, tag="pa")
            nc.sync.dma_start(out=pa, in_=pa_d[:, :])
            pb = sb.tile([128, WB], bf16, tag="pb")
            nc.scalar.dma_start(out=pb, in_=pb_d[:, :])

            w1lvT_a = pa[:, 0:128]
            w1muT_a = pa[:, 128:256]
            w1lvT_b = pa[64:128, 256:384]
            w1muT_b = pa[64:128, 384:512]
            xa0 = pa[:, 512:768]
            xb0 = pa[64:128, 768:1024]
            xa1 = pb[:, 0:256]
            xb1 = pb[64:128, 256:512]
            w2lvT = pb[:, 512:576]
            w2muT = pb[:, 576:640]
            # u_c: rows 0:64 = ysq, rows 64:128 = yd2 (per L half)
            u = [pb[:, 640:896], pb[:, 896:1152]]
            b1mu = pb[:, 1152:1154].bitcast(f32)
            b1lv = pb[:, 1154:1156].bitcast(f32)
            b2 = pb[:, 1156:1158].bitcast(f32)  # rows0:64 b2lv, rows64:128 b2mu
            mv = pb[:, 1158:1162].bitcast(f32)  # [128,2]: [±1 | -Ey2;0]

            # accumulator columns; cols 4:16 stay zero (collapse reads 0:16)
            acct = sb.tile([128, 16], f32, tag="acct")
            nc.gpsimd.memset(acct, 0.0)

            hs_lv = sb.tile([128, L], bf16, tag="hslv")
            hs_mu = sb.tile([128, L], bf16, tag="hsmu")
            ivd = sb.tile([128, L], f32, tag="ivd")

            mm = {}
            act_order = []
            dve_order = []
            gps_order = []
            for c in range(2):
                cs = slice(c * HC, (c + 1) * HC)
                xa_c = xa0 if c == 0 else xa1
                xb_c = xb0 if c == 0 else xb1
                # layer 1
                h_lv = ps.tile([128, HC], f32, tag=f"hlv{c}")
                mm[f"alv{c}"] = nc.tensor.matmul(
                    h_lv, w1lvT_a, xa_c, start=True, stop=False
                )
                mm[f"blv{c}"] = nc.tensor.matmul(
                    h_lv, w1lvT_b, xb_c, start=False, stop=True
                )
                h_mu = ps.tile([128, HC], f32, tag=f"hmu{c}")
                mm[f"amu{c}"] = nc.tensor.matmul(
                    h_mu, w1muT_a, xa_c, start=True, stop=False
                )
                mm[f"bmu{c}"] = nc.tensor.matmul(
                    h_mu, w1muT_b, xb_c, start=False, stop=True
                )
                # relu_lv on ACT, relu_mu on DVE
                act_order.append(
                    nc.scalar.activation(
                        out=hs_lv[:, cs], in_=h_lv, func=AF.Relu, bias=b1lv, scale=1.0
                    )
                )
                dve_order.append(
                    nc.vector.tensor_scalar(
                        out=hs_mu[:, cs], in0=h_mu, scalar1=b1mu, scalar2=0.0,
                        op0=OP.add, op1=OP.max,
                    )
                )
                # layer 2: one [128, HC] psum tile, lv rows 0:64, mu rows 64:128
                nb = ps.tile([128, HC], f32, tag=f"nb{c}")
                mm[f"w2lv{c}"] = nc.tensor.matmul(
                    nb[0:64, :], w2lvT, hs_lv[:, cs], start=True, stop=True
                )
                mm[f"w2mu{c}"] = nc.tensor.matmul(
                    nb[64:128, :], w2muT, hs_mu[:, cs], start=True, stop=True
                )
                # lv tail on ACT: tanh(+b2lv) -> exp(-.) with sum_l iv accum
                t1 = sb.tile([64, HC], f32, tag=f"t1{c}")
                act_order.append(
                    nc.scalar.activation(
                        out=t1, in_=nb[0:64, :], func=AF.Tanh, bias=b2[0:64, :],
                        scale=1.0,
                    )
                )
                act_order.append(
                    nc.scalar.activation(
                        out=ivd[0:64, cs], in_=t1, func=AF.Exp, scale=-1.0,
                        accum_out=acct[0:64, 2 + c : 3 + c],
                    )
                )
                # duplicate iv into rows 64:128 (for the stacked fin op)
                gps_order.append(
                    nc.gpsimd.tensor_copy(ivd[64:128, cs], ivd[0:64, cs])
                )
                # m2 = (mu_nb + b2mu) * yd2, in place over yd2 (rows 64:128)
                dve_order.append(
                    nc.vector.scalar_tensor_tensor(
                        out=u[c][64:128, :], in0=nb[64:128, :], scalar=b2[64:128, :],
                        in1=u[c][64:128, :], op0=OP.add, op1=OP.mult,
                    )
                )
                # fin = [ysq; m2] * [iv; iv], per-partition accumulators
                dve_order.append(
                    nc.vector.scalar_tensor_tensor(
                        out=ivd[:, cs], in0=u[c], scalar=1.0, in1=ivd[:, cs],
                        op0=OP.mult, op1=OP.mult, accum_out=acct[:, c : c + 1],
                    )
                )

            # PE stream order: half-1 L1 matmuls fill gaps while half-0
            # relus run on ACT/DVE
            pe_order = [
                mm["alv0"], mm["blv0"], mm["amu0"], mm["bmu0"],
                mm["alv1"], mm["w2lv0"], mm["blv1"], mm["w2mu0"],
                mm["amu1"], mm["bmu1"], mm["w2lv1"], mm["w2mu1"],
            ]
            # DVE stream: relu_mu0, m2_0, relu_mu1, fin_0, m2_1, fin_1
            dve_order = [
                dve_order[0], dve_order[1], dve_order[3],
                dve_order[2], dve_order[4], dve_order[5],
            ]
            # collapse: out[i,j] = sum_p acct[p,i] * mv[p,j]
            out_ps = ps.tile([16, 2], f32, tag="outps")
            mm_acc = nc.tensor.matmul(
                out_ps, acct[:, 0:16], mv, start=True, stop=True
            )
            pe_order.append(mm_acc)
            for order in (pe_order, act_order, dve_order, gps_order):
                for a_i, b_i in zip(order[1:], order[:-1]):
                    add_dep_helper(a_i.ins, b_i.ins, sync=False, reason="stream-order")

            out_sb = sb.tile([16, 2], f32, tag="outsb")
            nc.scalar.activation(out=out_sb, in_=out_ps, func=AF.Copy)
            nc.sync.dma_start(out=acc_d[:, :], in_=out_sb)

    nc.compile()
    return nc


def pack_inputs(inputs: dict) -> list[dict]:
    import ml_dtypes

    bf = ml_dtypes.bfloat16
    x = np.asarray(inputs["x_samples"], dtype=np.float32)
    y = np.ascontiguousarray(np.asarray(inputs["y_samples"], dtype=np.float32))
    mu_W1 = np.asarray(inputs["mu_W1"], dtype=np.float32)
    mu_b1 = np.asarray(inputs["mu_b1"], dtype=np.float32)
    mu_W2 = np.asarray(inputs["mu_W2"], dtype=np.float32)
    mu_b2 = np.asarray(inputs["mu_b2"], dtype=np.float32)
    lv_W1 = np.asarray(inputs["lv_W1"], dtype=np.float32)
    lv_b1 = np.asarray(inputs["lv_b1"], dtype=np.float32)
    lv_W2 = np.asarray(inputs["lv_W2"], dtype=np.float32)
    lv_b2 = np.asarray(inputs["lv_b2"], dtype=np.float32)

    def f32cols(a):  # [P] f32 -> [P, 2] bf16 raw-bit view
        return np.ascontiguousarray(a.astype(np.float32)[:, None]).view(bf)

    w1muT = mu_W1.T  # [192, 128]
    w1lvT = lv_W1.T
    x16 = x.astype(bf)

    # core-independent parts of pkA
    pa_base = np.zeros((128, WA), bf)
    pa_base[:, 0:128] = w1lvT[0:128].astype(bf)
    pa_base[:, 128:256] = w1muT[0:128].astype(bf)
    pa_base[64:128, 256:384] = w1lvT[128:192].astype(bf)
    pa_base[64:128, 384:512] = w1muT[128:192].astype(bf)

    # core-independent parts of pkB
    pb_base = np.zeros((128, WB), bf)
    pb_base[:, 512:576] = lv_W2.T.astype(bf)
    pb_base[:, 576:640] = mu_W2.T.astype(bf)
    pb_base[:, 1152:1154] = f32cols(np.broadcast_to(mu_b1, (128,)))
    pb_base[:, 1154:1156] = f32cols(np.broadcast_to(lv_b1, (128,)))
    b2col = np.concatenate([lv_b2, mu_b2])  # rows0:64 b2lv, rows64:128 b2mu
    pb_base[:, 1156:1158] = f32cols(b2col)
    sgn = np.concatenate([np.ones(64, np.float32), -np.ones(64, np.float32)])
    pb_base[:, 1158:1160] = f32cols(sgn)

    in_maps = []
    for bi in range(NCORES):
        pa = pa_base.copy()
        pa[:, 512:768] = x16[bi, 0:128, 0:256]
        pa[64:128, 768:1024] = x16[bi, 128:192, 0:256]

        pb = pb_base.copy()
        pb[:, 0:256] = x16[bi, 0:128, 256:512]
        pb[64:128, 256:512] = x16[bi, 128:192, 256:512]
        yb = y[bi]  # [64, 512]
        ey = yb.mean(axis=1)
        ysq = yb * yb
        ey2 = ysq.mean(axis=1)
        yd2 = 2.0 * (yb - ey[:, None])
        pb[0:64, 640:896] = ysq[:, 0:256].astype(bf)
        pb[64:128, 640:896] = yd2[:, 0:256].astype(bf)
        pb[0:64, 896:1152] = ysq[:, 256:512].astype(bf)
        pb[64:128, 896:1152] = yd2[:, 256:512].astype(bf)
        mv1 = np.concatenate([-ey2, np.zeros(64, np.float32)])
        pb[:, 1160:1162] = f32cols(mv1)

        in_maps.append({"pa": pa, "pb": pb})
    return in_maps


def _combine(results) -> float:
    tot = 0.0
    for r in results:
        a = r["acc"].astype(np.float64)  # [16, 2]
        # col0 rows 0,1: signed fin sums (ysq*iv - m2*iv) per half
        # col1 rows 2,3: -sum_d Ey2_d * sum_l iv per half
        tot += a[0, 0] + a[1, 0] + a[2, 1] + a[3, 1]
    return tot


def kernel(**inputs) -> np.ndarray:
    from concourse.bass_utils import run_bass_kernel_spmd

    if "nc" not in _CACHE:
        _CACHE["nc"] = build_nc(debug=False)
    nc = _CACHE["nc"]

    in_maps = pack_inputs(inputs)
    res = run_bass_kernel_spmd(nc, in_maps, core_ids=list(range(NCORES)))
    loss = -0.5 * _combine(res.results) / (B * L)
    return np.array(loss, dtype=np.float32)


# revision 3
# speedup vs baseline: 1.1625x; 1.1625x over previous
"""CLUB loss kernel for 8x TRN2 NeuronCores.

Math: per sample b (L=512 positions, D=64 dims):
  mu     = MLP_mu(x);  logvar = tanh(MLP_lv(x));  iv = exp(-logvar)
  loss = -0.5/(B*L) * sum_{b,d,l} [ ((ysq - Ey2) - mu*yd2) * iv ]
with ysq = y^2, yd2 = 2*(y - Ey); Ey/Ey2 per-(b,d) means over l.

y never feeds a matmul, so ysq/yd2/Ey2 are precomputed host-side and
shipped (bf16) instead of y.

Layout trick: everything after layer 1 runs in a (d, L-half) stacked
layout — partition p<64 is (d=p, half 0), p>=64 is (d=p-64, half 1).
The two w2 matmuls per path write one [128, 256] PSUM tile (separate
tiles for lv and mu so reads don't pick up false cross-path deps), so
tanh/exp/m2/v/fin are each ONE full-width op instead of two per-half
ops, and exp's accumulator gives sum_l iv for both halves in one
[128,1] column. fin = (ysq - m2) * iv accumulates per-partition; one
fp32 collapse matmul acct^T @ [1 | -Ey2dup] yields the 2 scalars the
host sums: loss_core = out[0,0] + out[1,1].

The w1 tail stationaries (channels 128:192) are shipped twice (rows
0:64 and 64:128) so the xb L-halves stack on aligned partitions with
zero padding waste.

All matmul operands bf16 (fp32 PE is 4 cyc/col; fp32r truncates to
~bf16 anyway); PSUM/elementwise f32. bf16 ysq/yd2/v add ~1e-4 rel err.

DMA: two packed [128, W] bf16 inputs (SP and ACT HWDGE triggers, 128
descriptors each, no SWDGE). f32 consts (biases, collapse vectors)
ride in the packs and are bitcast on-chip. Output store is [16, 2] —
measured floor tests showed 16 small descriptors beat both
single-packet (lazy completion sems) and 128-descriptor stores.

Sharding: data-parallel over batch B=8, one sample per core; host
does the tiny final combine.
"""

import sys

if "/opt/trn_rl_repo" not in sys.path:
    sys.path.insert(0, "/opt/trn_rl_repo")

import numpy as np

B, L = 8, 512
XD, YD, H = 192, 64, 128
NCORES = 8
HC = L // 2

WA = 1024
WB = 908

_CACHE: dict = {}


def build_nc(debug: bool = False):
    import concourse.bass as bass
    import concourse.bacc as bacc
    import concourse.tile as tile
    from concourse import mybir
    from concourse.tile import add_dep_helper

    f32 = mybir.dt.float32
    bf16 = mybir.dt.bfloat16
    AF = mybir.ActivationFunctionType
    OP = mybir.AluOpType

    nc = bacc.Bacc("TRN2", target_bir_lowering=False, debug=debug)

    pa_d = nc.dram_tensor("pa", [128, WA], bf16, kind="ExternalInput")
    pb_d = nc.dram_tensor("pb", [128, WB], bf16, kind="ExternalInput")
    acc_d = nc.dram_tensor("acc", [16, 2], f32, kind="ExternalOutput")

    with tile.TileContext(nc) as tc:
        with (
            tc.tile_pool(name="sb", bufs=1) as sb,
            tc.tile_pool(name="ps", bufs=1, space=bass.MemorySpace.PSUM) as ps,
        ):
            pa = sb.tile([128, WA], bf16, tag="pa")
            nc.sync.dma_start(out=pa, in_=pa_d[:, :])
            pb = sb.tile([128, WB], bf16, tag="pb")
            nc.scalar.dma_start(out=pb, in_=pb_d[:, :])

            w1lvT_a = pa[:, 0:128]
            w1muT_a = pa[:, 128:256]
            w1lvT_b = pa[:, 256:384]   # rows 0:64 and 64:128 hold same data
            w1muT_b = pa[:, 384:512]
            xa0 = pa[:, 512:768]
            xbs = pa[:, 768:1024]      # rows 0:64 = xb half0, 64:128 = half1
            xa1 = pb[:, 0:256]
            w2lvT = pb[:, 256:320]
            w2muT = pb[:, 320:384]
            ysq2 = pb[:, 384:640]      # (d, half) stacked
            yd22 = pb[:, 640:896]
            b1mu = pb[:, 896:898].bitcast(f32)
            b1lv = pb[:, 898:900].bitcast(f32)
            b2lv = pb[:, 900:902].bitcast(f32)
            b2mu = pb[:, 902:904].bitcast(f32)
            mv = pb[:, 904:908].bitcast(f32)   # [128,2]: [1 | -Ey2 dup]

            acct = sb.tile([128, 16], f32, tag="acct")
            nc.gpsimd.memset(acct, 0.0)

            hs_lv = sb.tile([128, L], bf16, tag="hslv")
            hs_mu = sb.tile([128, L], bf16, tag="hsmu")
            tt = sb.tile([128, HC], f32, tag="tt")
            ivd = sb.tile([128, HC], f32, tag="ivd")

            # layer 1
            h_lv0 = ps.tile([128, HC], f32, tag="hlv0")
            h_lv1 = ps.tile([128, HC], f32, tag="hlv1")
            h_mu0 = ps.tile([128, HC], f32, tag="hmu0")
            h_mu1 = ps.tile([128, HC], f32, tag="hmu1")
            alv0 = nc.tensor.matmul(h_lv0, w1lvT_a, xa0, start=True, stop=False)
            blv0 = nc.tensor.matmul(
                h_lv0, w1lvT_b[0:64, :], xbs[0:64, :], start=False, stop=True
            )
            alv1 = nc.tensor.matmul(h_lv1, w1lvT_a, xa1, start=True, stop=False)
            blv1 = nc.tensor.matmul(
                h_lv1, w1lvT_b[64:128, :], xbs[64:128, :], start=False, stop=True
            )
            amu0 = nc.tensor.matmul(h_mu0, w1muT_a, xa0, start=True, stop=False)
            bmu0 = nc.tensor.matmul(
                h_mu0, w1muT_b[0:64, :], xbs[0:64, :], start=False, stop=True
            )
            amu1 = nc.tensor.matmul(h_mu1, w1muT_a, xa1, start=True, stop=False)
            bmu1 = nc.tensor.matmul(
                h_mu1, w1muT_b[64:128, :], xbs[64:128, :], start=False, stop=True
            )

            # relus (all ACT; DVE is reserved for the m2/v/fin tail)
            r_lv0 = nc.scalar.activation(
                out=hs_lv[:, 0:HC], in_=h_lv0, func=AF.Relu, bias=b1lv, scale=1.0
            )
            r_lv1 = nc.scalar.activation(
                out=hs_lv[:, HC:L], in_=h_lv1, func=AF.Relu, bias=b1lv, scale=1.0
            )
            r_mu0 = nc.scalar.activation(
                out=hs_mu[:, 0:HC], in_=h_mu0, func=AF.Relu, bias=b1mu, scale=1.0
            )
            r_mu1 = nc.scalar.activation(
                out=hs_mu[:, HC:L], in_=h_mu1, func=AF.Relu, bias=b1mu, scale=1.0
            )

            # layer 2 into (d, half) stacked PSUM tiles
            nbLV = ps.tile([128, HC], f32, tag="nblv")
            nbMU = ps.tile([128, HC], f32, tag="nbmu")
            w2lv0 = nc.tensor.matmul(
                nbLV[0:64, :], w2lvT, hs_lv[:, 0:HC], start=True, stop=True
            )
            w2lv1 = nc.tensor.matmul(
                nbLV[64:128, :], w2lvT, hs_lv[:, HC:L], start=True, stop=True
            )
            w2mu0 = nc.tensor.matmul(
                nbMU[0:64, :], w2muT, hs_mu[:, 0:HC], start=True, stop=True
            )
            w2mu1 = nc.tensor.matmul(
                nbMU[64:128, :], w2muT, hs_mu[:, HC:L], start=True, stop=True
            )

            # lv tail: tanh(+b2lv) -> exp(-.) with sum_l iv accumulator
            a_tanh = nc.scalar.activation(
                out=tt, in_=nbLV, func=AF.Tanh, bias=b2lv, scale=1.0
            )
            a_exp = nc.scalar.activation(
                out=ivd, in_=tt, func=AF.Exp, scale=-1.0,
                accum_out=acct[:, 1:2],
            )

            # mu tail on DVE: m2 = (nbMU + b2mu)*yd2 (in place over yd2),
            # v = ysq - m2 (in place over ysq), fin = v*iv (accum col 0)
            d_m2 = nc.vector.scalar_tensor_tensor(
                out=yd22, in0=nbMU, scalar=b2mu, in1=yd22,
                op0=OP.add, op1=OP.mult,
            )
            d_v = nc.vector.tensor_tensor(
                out=ysq2, in0=ysq2, in1=yd22, op=OP.subtract
            )
            d_fin = nc.vector.scalar_tensor_tensor(
                out=ivd, in0=ysq2, scalar=1.0, in1=ivd,
                op0=OP.mult, op1=OP.mult, accum_out=acct[:, 0:1],
            )

            out_ps = ps.tile([16, 2], f32, tag="outps")
            mm_acc = nc.tensor.matmul(out_ps, acct[:, 0:16], mv, start=True, stop=True)

            pe_order = [
                alv0, blv0, alv1, blv1, amu0, bmu0, amu1, bmu1,
                w2lv0, w2mu0, w2lv1, w2mu1, mm_acc,
            ]
            act_order = [r_lv0, r_lv1, r_mu0, r_mu1, a_tanh, a_exp]
            dve_order = [d_m2, d_v, d_fin]
            for order in (pe_order, act_order, dve_order):
                for a_i, b_i in zip(order[1:], order[:-1]):
                    add_dep_helper(a_i.ins, b_i.ins, sync=False, reason="stream-order")

            out_sb = sb.tile([16, 2], f32, tag="outsb")
            nc.vector.tensor_copy(out_sb, out_ps)
            nc.sync.dma_start(out=acc_d[:, :], in_=out_sb)

    nc.compile()
    return nc


def pack_inputs(inputs: dict) -> list[dict]:
    import ml_dtypes

    bf = ml_dtypes.bfloat16
    x = np.asarray(inputs["x_samples"], dtype=np.float32)
    y = np.ascontiguousarray(np.asarray(inputs["y_samples"], dtype=np.float32))
    mu_W1 = np.asarray(inputs["mu_W1"], dtype=np.float32)
    mu_b1 = np.asarray(inputs["mu_b1"], dtype=np.float32)
    mu_W2 = np.asarray(inputs["mu_W2"], dtype=np.float32)
    mu_b2 = np.asarray(inputs["mu_b2"], dtype=np.float32)
    lv_W1 = np.asarray(inputs["lv_W1"], dtype=np.float32)
    lv_b1 = np.asarray(inputs["lv_b1"], dtype=np.float32)
    lv_W2 = np.asarray(inputs["lv_W2"], dtype=np.float32)
    lv_b2 = np.asarray(inputs["lv_b2"], dtype=np.float32)

    def f32cols(a):  # [P] f32 -> [P, 2] bf16 raw-bit view
        return np.ascontiguousarray(a.astype(np.float32)[:, None]).view(bf)

    w1muT = mu_W1.T  # [192, 128]
    w1lvT = lv_W1.T
    x16 = x.astype(bf)
    w1lvT_b = w1lvT[128:192].astype(bf)
    w1muT_b = w1muT[128:192].astype(bf)

    pa_base = np.zeros((128, WA), bf)
    pa_base[:, 0:128] = w1lvT[0:128].astype(bf)
    pa_base[:, 128:256] = w1muT[0:128].astype(bf)
    pa_base[0:64, 256:384] = w1lvT_b
    pa_base[64:128, 256:384] = w1lvT_b
    pa_base[0:64, 384:512] = w1muT_b
    pa_base[64:128, 384:512] = w1muT_b

    pb_base = np.zeros((128, WB), bf)
    pb_base[:, 256:320] = lv_W2.T.astype(bf)
    pb_base[:, 320:384] = mu_W2.T.astype(bf)
    pb_base[:, 896:898] = f32cols(np.broadcast_to(mu_b1, (128,)))
    pb_base[:, 898:900] = f32cols(np.broadcast_to(lv_b1, (128,)))
    pb_base[:, 900:902] = f32cols(np.tile(lv_b2, 2))
    pb_base[:, 902:904] = f32cols(np.tile(mu_b2, 2))
    pb_base[:, 904:906] = f32cols(np.ones(128, np.float32))

    in_maps = []
    for bi in range(NCORES):
        pa = pa_base.copy()
        pa[:, 512:768] = x16[bi, 0:128, 0:256]
        pa[0:64, 768:1024] = x16[bi, 128:192, 0:256]
        pa[64:128, 768:1024] = x16[bi, 128:192, 256:512]

        pb = pb_base.copy()
        pb[:, 0:256] = x16[bi, 0:128, 256:512]
        yb = y[bi]  # [64, 512]
        ey = yb.mean(axis=1)
        ysq = yb * yb
        ey2 = ysq.mean(axis=1)
        yd2 = 2.0 * (yb - ey[:, None])
        pb[0:64, 384:640] = ysq[:, 0:256].astype(bf)
        pb[64:128, 384:640] = ysq[:, 256:512].astype(bf)
        pb[0:64, 640:896] = yd2[:, 0:256].astype(bf)
        pb[64:128, 640:896] = yd2[:, 256:512].astype(bf)
        pb[:, 906:908] = f32cols(np.tile(-ey2, 2))

        in_maps.append({"pa": pa, "pb": pb})
    return in_maps


def _combine(results) -> float:
    tot = 0.0
    for r in results:
        a = r["acc"].astype(np.float64)  # [16, 2]
        # out[0,0] = sum (ysq - m2)*iv ; out[1,1] = -sum_d Ey2_d * sum_l iv
        tot += a[0, 0] + a[1, 1]
    return tot


def kernel(**inputs) -> np.ndarray:
    from concourse.bass_utils import run_bass_kernel_spmd

    if "nc" not in _CACHE:
        _CACHE["nc"] = build_nc(debug=False)
    nc = _CACHE["nc"]

    in_maps = pack_inputs(inputs)
    res = run_bass_kernel_spmd(nc, in_maps, core_ids=list(range(NCORES)))
    loss = -0.5 * _combine(res.results) / (B * L)
    return np.array(loss, dtype=np.float32)


# revision 4
# speedup vs baseline: 1.3032x; 1.1210x over previous
"""CLUB loss kernel for 8x TRN2 NeuronCores.

Math: per sample b (L=512 positions, D=64 dims):
  mu     = MLP_mu(x);  logvar = tanh(MLP_lv(x));  iv = exp(-logvar)
  loss = -0.5/(B*L) * sum_{b,d,l} [ ((ysq - Ey2) - mu*yd2) * iv ]
with ysq = y^2, yd2 = 2*(y - Ey); Ey/Ey2 per-(b,d) means over l.

y never feeds a matmul, so ysq/yd2/Ey2 are precomputed host-side and
shipped (bf16) instead of y.

Layer 1 runs in fp8 e4m3 DoubleRow mode (2 MACs/PE-row/cycle): the
192-channel contraction packs as 96 partitions x 2 rows, so one
matmul per (path, L-half) replaces the bf16 a/b split pair and the PE
spine halves. w1 ships x8 (lifting ~N(0,0.05) weights out of the e4m3
subnormal range); relu is positive-homogeneous so hs = relu(8h + 8*b1)
and the 8x cancels via w2/8 shipped host-side. Quantization errors are
random-sign across 32K summed terms; measured end-to-end error stays
~1e-3.

Everything after layer 1 runs in a (d, L-half) stacked layout -
partition p<64 is (d=p, half 0), p>=64 is (d=p-64, half 1) - so
tanh/exp/m2/v/fin are single full-width [128, 256] ops. Ey2 is folded
into the final DVE op: fin = ((ysq - m2) - Ey2)*iv accumulated
per-partition, so the scalar loss needs only one ones-vector collapse
matmul and no ACT accumulator on the critical path.

DMA: two packed [128, W] bf16-typed inputs (fp8/f32 regions ride in
them and are bitcast on-chip), triggered on SP and ACT HWDGE, 128
descriptors each. Output store is [4, 1] f32.

Sharding: data-parallel over batch B=8, one sample per core; host
does the tiny final combine.
"""

import sys

if "/opt/trn_rl_repo" not in sys.path:
    sys.path.insert(0, "/opt/trn_rl_repo")

import numpy as np

B, L = 8, 512
XD, YD, H = 192, 64, 128
NCORES = 8
HC = L // 2

WA = 768   # bf16-cols: w1lv8 128 | w1mu8 128 | x8 half0 256 | x8 half1 256
WB = 652   # xa-free: w2lv 64 | w2mu 64 | ysq2 256 | yd22 256 | consts 12

_CACHE: dict = {}


def build_nc(debug: bool = False):
    import concourse.bass as bass
    import concourse.bacc as bacc
    import concourse.tile as tile
    from concourse import mybir
    from concourse.tile import add_dep_helper

    f32 = mybir.dt.float32
    bf16 = mybir.dt.bfloat16
    f8 = mybir.dt.float8e4
    AF = mybir.ActivationFunctionType
    OP = mybir.AluOpType
    DR = mybir.MatmulPerfMode.DoubleRow

    nc = bacc.Bacc("TRN2", target_bir_lowering=False, debug=debug)

    pa_d = nc.dram_tensor("pa", [128, WA], bf16, kind="ExternalInput")
    pb_d = nc.dram_tensor("pb", [128, WB], bf16, kind="ExternalInput")
    acc_d = nc.dram_tensor("acc", [4, 1], f32, kind="ExternalOutput")

    with tile.TileContext(nc) as tc:
        with (
            tc.tile_pool(name="sb", bufs=1) as sb,
            tc.tile_pool(name="ps", bufs=1, space=bass.MemorySpace.PSUM) as ps,
        ):
            pa = sb.tile([128, WA], bf16, tag="pa")
            nc.sync.dma_start(out=pa, in_=pa_d[:, :])
            pb = sb.tile([128, WB], bf16, tag="pb")
            nc.scalar.dma_start(out=pb, in_=pb_d[:, :])

            def dr3(ap, m):  # [96, 2m fp8] -> [96, 2, m] DoubleRow operand
                return ap.bitcast(f8).rearrange("p (two f) -> p two f", two=2)

            w1lv8 = dr3(pa[0:96, 0:128], 128)     # [96, 2, 128]
            w1mu8 = dr3(pa[0:96, 128:256], 128)
            x8 = [dr3(pa[0:96, 256:512], 256), dr3(pa[0:96, 512:768], 256)]
            w2lvT = pb[:, 0:64]    # w2/8, bf16
            w2muT = pb[:, 64:128]
            ysq2 = pb[:, 128:384]      # (d, half) stacked
            yd22 = pb[:, 384:640]
            b1lv8 = pb[:, 640:642].bitcast(f32)   # 8*b1
            b1mu8 = pb[:, 642:644].bitcast(f32)
            b2lv = pb[:, 644:646].bitcast(f32)    # rows duplicated per half
            b2mu = pb[:, 646:648].bitcast(f32)
            ey2c = pb[:, 648:650].bitcast(f32)    # Ey2 dup
            ones = pb[:, 650:652].bitcast(f32)

            acct = sb.tile([128, 4], f32, tag="acct")
            nc.gpsimd.memset(acct, 0.0)

            hs_lv = sb.tile([128, L], bf16, tag="hslv")
            hs_mu = sb.tile([128, L], bf16, tag="hsmu")
            tt = sb.tile([128, HC], f32, tag="tt")
            ivd = sb.tile([128, HC], f32, tag="ivd")

            # layer 1: fp8 DoubleRow, one matmul per (path, half)
            h_lv0 = ps.tile([128, HC], f32, tag="hlv0")
            h_lv1 = ps.tile([128, HC], f32, tag="hlv1")
            h_mu0 = ps.tile([128, HC], f32, tag="hmu0")
            h_mu1 = ps.tile([128, HC], f32, tag="hmu1")
            dlv0 = nc.tensor.matmul(h_lv0, w1lv8, x8[0], start=True, stop=True,
                                    perf_mode=DR)
            dlv1 = nc.tensor.matmul(h_lv1, w1lv8, x8[1], start=True, stop=True,
                                    perf_mode=DR)
            dmu0 = nc.tensor.matmul(h_mu0, w1mu8, x8[0], start=True, stop=True,
                                    perf_mode=DR)
            dmu1 = nc.tensor.matmul(h_mu1, w1mu8, x8[1], start=True, stop=True,
                                    perf_mode=DR)

            # relus emit 8*relu(h + b1); the 8x cancels in w2/8.
            # relu_mu1 runs on DVE to keep the ACT spine short.
            r_lv0 = nc.scalar.activation(
                out=hs_lv[:, 0:HC], in_=h_lv0, func=AF.Relu, bias=b1lv8, scale=1.0
            )
            r_lv1 = nc.scalar.activation(
                out=hs_lv[:, HC:L], in_=h_lv1, func=AF.Relu, bias=b1lv8, scale=1.0
            )
            r_mu0 = nc.scalar.activation(
                out=hs_mu[:, 0:HC], in_=h_mu0, func=AF.Relu, bias=b1mu8, scale=1.0
            )
            r_mu1 = nc.vector.tensor_scalar(
                out=hs_mu[:, HC:L], in0=h_mu1, scalar1=b1mu8, scalar2=0.0,
                op0=OP.add, op1=OP.max,
            )

            # layer 2 (bf16) into (d, half) stacked PSUM tiles
            nbLV = ps.tile([128, HC], f32, tag="nblv")
            nbMU = ps.tile([128, HC], f32, tag="nbmu")
            w2lv0 = nc.tensor.matmul(
                nbLV[0:64, :], w2lvT, hs_lv[:, 0:HC], start=True, stop=True
            )
            w2lv1 = nc.tensor.matmul(
                nbLV[64:128, :], w2lvT, hs_lv[:, HC:L], start=True, stop=True
            )
            w2mu0 = nc.tensor.matmul(
                nbMU[0:64, :], w2muT, hs_mu[:, 0:HC], start=True, stop=True
            )
            w2mu1 = nc.tensor.matmul(
                nbMU[64:128, :], w2muT, hs_mu[:, HC:L], start=True, stop=True
            )

            # lv tail: tanh(+b2lv) -> exp(-.)
            a_tanh = nc.scalar.activation(
                out=tt, in_=nbLV, func=AF.Tanh, bias=b2lv, scale=1.0
            )
            a_exp = nc.scalar.activation(
                out=ivd, in_=tt, func=AF.Exp, scale=-1.0
            )

            # mu tail on DVE: m2 = (nbMU + b2mu)*yd2 (in place over yd2),
            # v = ysq - m2 (in place over ysq),
            # fin = (v - Ey2)*iv accumulated per partition into acct col 0
            d_m2 = nc.vector.scalar_tensor_tensor(
                out=yd22, in0=nbMU, scalar=b2mu, in1=yd22,
                op0=OP.add, op1=OP.mult,
            )
            d_v = nc.vector.tensor_tensor(
                out=ysq2, in0=ysq2, in1=yd22, op=OP.subtract
            )
            d_fin = nc.vector.scalar_tensor_tensor(
                out=ivd, in0=ysq2, scalar=ey2c, in1=ivd,
                op0=OP.subtract, op1=OP.mult, accum_out=acct[:, 0:1],
            )

            out_ps = ps.tile([4, 1], f32, tag="outps")
            mm_acc = nc.tensor.matmul(out_ps, acct[:, 0:4], ones, start=True, stop=True)

            pe_order = [
                dlv0, dlv1, dmu0, dmu1,
                w2lv0, w2lv1, w2mu0, w2mu1, mm_acc,
            ]
            act_order = [r_lv0, r_lv1, r_mu0, a_tanh, a_exp]
            dve_order = [r_mu1, d_m2, d_v, d_fin]
            for order in (pe_order, act_order, dve_order):
                for a_i, b_i in zip(order[1:], order[:-1]):
                    add_dep_helper(a_i.ins, b_i.ins, sync=False, reason="stream-order")

            out_sb = sb.tile([4, 1], f32, tag="outsb")
            nc.vector.tensor_copy(out_sb, out_ps)
            nc.sync.dma_start(out=acc_d[:, :], in_=out_sb)

    nc.compile()
    return nc


def pack_inputs(inputs: dict) -> list[dict]:
    import ml_dtypes

    bf = ml_dtypes.bfloat16
    f8 = ml_dtypes.float8_e4m3fn
    x = np.asarray(inputs["x_samples"], dtype=np.float32)
    y = np.ascontiguousarray(np.asarray(inputs["y_samples"], dtype=np.float32))
    mu_W1 = np.asarray(inputs["mu_W1"], dtype=np.float32)
    mu_b1 = np.asarray(inputs["mu_b1"], dtype=np.float32)
    mu_W2 = np.asarray(inputs["mu_W2"], dtype=np.float32)
    mu_b2 = np.asarray(inputs["mu_b2"], dtype=np.float32)
    lv_W1 = np.asarray(inputs["lv_W1"], dtype=np.float32)
    lv_b1 = np.asarray(inputs["lv_b1"], dtype=np.float32)
    lv_W2 = np.asarray(inputs["lv_W2"], dtype=np.float32)
    lv_b2 = np.asarray(inputs["lv_b2"], dtype=np.float32)

    def f32cols(a):  # [P] f32 -> [P, 2] bf16 raw-bit view
        return np.ascontiguousarray(a.astype(np.float32)[:, None]).view(bf)

    def drpack(wT8):  # [192, M] -> [96, 2M] fp8 bytes (DoubleRow groups)
        k, m = 96, wT8.shape[1]
        out = np.zeros((96, 2 * m), f8)
        out[:, 0:m] = wT8[0:96]
        out[:, m : 2 * m] = wT8[96:192]
        return out.view(np.uint8)

    # pa: byte-level build (fp8 payload), viewed as bf16 at the end
    pa_bytes_base = np.zeros((128, WA * 2), np.uint8)
    w1lv8 = (lv_W1.T * 8.0).astype(f8)  # [192, 128]
    w1mu8 = (mu_W1.T * 8.0).astype(f8)
    pa_bytes_base[0:96, 0:256] = drpack(w1lv8)
    pa_bytes_base[0:96, 256:512] = drpack(w1mu8)

    pb_base = np.zeros((128, WB), bf)
    pb_base[:, 0:64] = (lv_W2.T / 8.0).astype(bf)
    pb_base[:, 64:128] = (mu_W2.T / 8.0).astype(bf)
    pb_base[:, 640:642] = f32cols(np.broadcast_to(8.0 * lv_b1, (128,)))
    pb_base[:, 642:644] = f32cols(np.broadcast_to(8.0 * mu_b1, (128,)))
    pb_base[:, 644:646] = f32cols(np.tile(lv_b2, 2))
    pb_base[:, 646:648] = f32cols(np.tile(mu_b2, 2))
    pb_base[:, 650:652] = f32cols(np.ones(128, np.float32))

    x8 = x.astype(f8).view(np.uint8)  # [B, 192, 512]

    in_maps = []
    for bi in range(NCORES):
        pa_bytes = pa_bytes_base.copy()
        for c in range(2):
            cs = slice(512 + 512 * c, 1024 + 512 * c)
            ls = slice(256 * c, 256 * c + 256)
            pa_bytes[0:96, cs.start : cs.start + 256] = x8[bi, 0:96, ls]
            pa_bytes[0:96, cs.start + 256 : cs.stop] = x8[bi, 96:192, ls]
        pa = pa_bytes.view(bf)

        pb = pb_base.copy()
        yb = y[bi]  # [64, 512]
        ey = yb.mean(axis=1)
        ysq = yb * yb
        ey2 = ysq.mean(axis=1)
        yd2 = 2.0 * (yb - ey[:, None])
        pb[0:64, 128:384] = ysq[:, 0:256].astype(bf)
        pb[64:128, 128:384] = ysq[:, 256:512].astype(bf)
        pb[0:64, 384:640] = yd2[:, 0:256].astype(bf)
        pb[64:128, 384:640] = yd2[:, 256:512].astype(bf)
        pb[:, 648:650] = f32cols(np.tile(ey2, 2))

        in_maps.append({"pa": pa, "pb": pb})
    return in_maps


def _combine(results) -> float:
    tot = 0.0
    for r in results:
        a = r["acc"].astype(np.float64)  # [4, 1]
        tot += a[0, 0]  # sum ((ysq - m2) - Ey2)*iv over (d, half, l)
    return tot


def kernel(**inputs) -> np.ndarray:
    from concourse.bass_utils import run_bass_kernel_spmd

    if "nc" not in _CACHE:
        _CACHE["nc"] = build_nc(debug=False)
    nc = _CACHE["nc"]

    in_maps = pack_inputs(inputs)
    res = run_bass_kernel_spmd(nc, in_maps, core_ids=list(range(NCORES)))
    loss = -0.5 * _combine(res.results) / (B * L)
    return np.array(loss, dtype=np.float32)
